# revision 37
# baseline (speedup 1.0000x reference)
"""Multi-head attention (B=2, S=2048, D=1024, H=16) on 8 trn2 NeuronCores.

Sharding: tensor-parallel over heads. Core c owns heads {2c, 2c+1} (128 of the
1024 projection output dims). Each core:
  1. computes qhT/khT/vh for its 128 head-dims (column-parallel QKV projections,
     contraction over full D with host-pretransposed activations),
  2. runs attention for its 4 (batch, head) pairs in two orientations:
       - transposed [key, query]: exp -> PV matmul with a ones-augmented V, so
         ctx.T and the softmax denominators fall out of one accumulation,
       - natural [query, key]: exp with bias = -ln(denominator) folded in,
         writing normalized attention probabilities (the `attn` output) with
         no separate normalization pass,
  3. computes its partial output projection (row-parallel Wo split).
Host sums the 8 partial outputs (the "all-reduce" of row-parallel TP), adds bo,
and concatenates attention shards over the head axis.

Matmuls run as float32r (fp32 rounded to 11 mantissa bits, full-rate on the PE
at free-dim >= 256). Host pre-rounds all DRAM-fed matmul operands; on-device
operands get rounded by the DVE/ACT op that produces them.

Emission is software-pipelined so the DMA engines never idle: projections
stream per-128/256-column chunk (attention starts as soon as chunk 0 lands),
the transposed pass for each (batch, head) runs before its natural pass, and
independent streams are interleaved:

  proj(b0) -> T(b0,j0)||proj(b1) -> T(b0,j1)||N(b0,j0) -> T(b1,j0)||N(b0,j1)
    -> T(b1,j1)||N(b1,j0) -> N(b1,j1)||O(b0) -> O(b1)
"""
import numpy as np
from contextlib import ExitStack

B = 2
S = 2048
NS = B * S          # 4096 flattened (b, s)
D = 1024
P = 128
KC = D // P         # contraction chunks for the projections
HD = 128            # head dims per core (2 heads x d_k 64)
NCORES = 8
CH = 256            # projection column-chunk
VSTRIDE = 66        # per-head block in the ones-augmented V tile
QT = S // P         # 16 query tiles per batch
KT = S // P         # 16 key tiles per batch

_cache = {}


def _round_f32r(x):
    b = np.ascontiguousarray(x, dtype=np.float32).view(np.uint32)
    return (((b.astype(np.uint64) + 0x800) & 0xFFFFF000).astype(np.uint32)).view(np.float32)


def _interleave(*gens_ratios, stop_on_first=False):
    """gens_ratios: (gen, ratio) pairs; pulls `ratio` units per round.
    stop_on_first: return as soon as any generator exhausts (others resumable)."""
    gens = [[g, r, False] for g, r in gens_ratios]
    while any(not g[2] for g in gens):
        for g in gens:
            if g[2]:
                continue
            for _ in range(g[1]):
                try:
                    next(g[0])
                except StopIteration:
                    g[2] = True
                    break
            if g[2] and stop_on_first:
                return


def build():
    import concourse.mybir as mybir
    import concourse.tile as tile
    from concourse import bacc
    from concourse.masks import make_identity

    f32 = mybir.dt.float32
    f32r = mybir.dt.float32r
    Exp = mybir.ActivationFunctionType.Exp

    nc = bacc.Bacc("TRN2", target_bir_lowering=False, debug=False, num_devices=NCORES)

    qT = nc.dram_tensor("qT", [D, NS], f32r, kind="ExternalInput").ap()
    kT = nc.dram_tensor("kT", [D, NS], f32r, kind="ExternalInput").ap()
    vT = nc.dram_tensor("vT", [D, NS], f32r, kind="ExternalInput").ap()
    wqT = nc.dram_tensor("wqT", [D, HD], f32r, kind="ExternalInput").ap()
    wkT = nc.dram_tensor("wkT", [D, HD], f32r, kind="ExternalInput").ap()
    wvT = nc.dram_tensor("wvT", [D, HD], f32r, kind="ExternalInput").ap()
    woT = nc.dram_tensor("woT", [HD, D], f32r, kind="ExternalInput").ap()
    bq = nc.dram_tensor("bq", [HD], f32, kind="ExternalInput").ap()
    bk = nc.dram_tensor("bk", [HD], f32, kind="ExternalInput").ap()
    bv = nc.dram_tensor("bv", [HD], f32, kind="ExternalInput").ap()
    attn_d = nc.dram_tensor("attn", [B, 2, S, S], f32, kind="ExternalOutput").ap()
    pout_d = nc.dram_tensor("pout", [NS, D], f32, kind="ExternalOutput").ap()

    with tile.TileContext(nc) as tc, ExitStack() as ctx:
        const = ctx.enter_context(tc.tile_pool(name="const", bufs=1))
        acts = ctx.enter_context(tc.tile_pool(name="acts", bufs=1))
        ld = ctx.enter_context(tc.tile_pool(name="ld", bufs=3))
        attnp = ctx.enter_context(tc.tile_pool(name="attnp", bufs=4))
        expp = ctx.enter_context(tc.tile_pool(name="expp", bufs=4))
        small = ctx.enter_context(tc.tile_pool(name="small", bufs=4))
        nlsp = ctx.enter_context(tc.tile_pool(name="nlsp", bufs=2))
        ctxs_pool = ctx.enter_context(tc.tile_pool(name="ctxs", bufs=2))
        sc = ctx.enter_context(tc.tile_pool(name="sc", bufs=3, space="PSUM"))
        ctxp = ctx.enter_context(tc.tile_pool(name="ctxp", bufs=2, space="PSUM"))
        dscr = ctx.enter_context(tc.tile_pool(name="dscr", bufs=2, space="DRAM"))

        wq = const.tile([P, KC, HD], f32r)
        wk = const.tile([P, KC, HD], f32r)
        wv = const.tile([P, KC, HD], f32r)
        nc.sync.dma_start(wq, wqT.rearrange("(kc p) m -> p kc m", p=P))
        nc.sync.dma_start(wk, wkT.rearrange("(kc p) m -> p kc m", p=P))
        nc.sync.dma_start(wv, wvT.rearrange("(kc p) m -> p kc m", p=P))
        wo = const.tile([P, D], f32r)
        nc.sync.dma_start(wo, woT)
        bqs = const.tile([P, 1], f32)
        bks = const.tile([P, 1], f32)
        bvs = const.tile([P, 1], f32)
        nc.sync.dma_start(bqs, bq.rearrange("(p o) -> p o", o=1))
        nc.sync.dma_start(bks, bk.rearrange("(p o) -> p o", o=1))
        nc.sync.dma_start(bvs, bv.rearrange("(p o) -> p o", o=1))
        ident = const.tile([P, P], f32)
        make_identity(nc, ident)

        # per-batch activation tiles so attention(b) only waits on proj(b)
        qhT = [acts.tile([P, S], f32r, name=f"qhT{b}") for b in range(B)]
        khT = [acts.tile([P, S], f32r, name=f"khT{b}") for b in range(B)]
        vh = [acts.tile([P, KT, VSTRIDE * 2], f32r, name=f"vh{b}") for b in range(B)]
        for b in range(B):
            nc.vector.memset(vh[b][:, :, 64:65].bitcast(f32), 1.0)
            nc.vector.memset(vh[b][:, :, VSTRIDE + 64:VSTRIDE + 65].bitcast(f32), 1.0)

        ctxTs = [None, None]
        nlts = [[None, None], [None, None]]  # -ln(rowsum) as [P, QT] per (b, j)

        vhT1 = acts.tile([P, S], f32, name="vhT1")  # staging for detached b1 v-transposes

        def gen_proj(b):
            """Stream projections per CH-column chunk, q/k/v grouped per chunk
            so attention T(b) can start as soon as chunk 0 lands. For b=1 the
            v transposes are detached (gen_vtrans) so late-arriving v chunks
            never sit ahead of next-phase work in the PE FIFO."""
            hch = S // CH
            for tt in range(hch):
                nsl_g = slice(b * S + tt * CH, b * S + (tt + 1) * CH)
                nsl_l = slice(tt * CH, (tt + 1) * CH)
                for kind in ("v", "q", "k"):
                    src, wt, bs = {"q": (qT, wq, bqs), "k": (kT, wk, bks),
                                   "v": (vT, wv, bvs)}[kind]
                    xt = ld.tile([P, KC, CH], f32r, tag="ld")
                    nc.sync.dma_start(xt, src.rearrange("(kc p) n -> p kc n", p=P)[:, :, nsl_g])
                    ps = sc.tile([P, 1024], f32, tag="sc")
                    for kc in range(KC):
                        nc.tensor.matmul(ps[:, :CH], lhsT=wt[:, kc, :], rhs=xt[:, kc, :],
                                         start=(kc == 0), stop=(kc == KC - 1))
                    if kind == "q":
                        nc.vector.tensor_scalar_add(qhT[b][:, nsl_l], ps[:, :CH], bqs)
                    elif kind == "k":
                        nc.vector.tensor_scalar_add(khT[b][:, nsl_l], ps[:, :CH], bks)
                    elif b == 1:
                        nc.vector.tensor_scalar_add(vhT1[:, nsl_l], ps[:, :CH], bvs)
                    else:
                        vt_tmp = ld.tile([P, CH], f32, tag="vtmp")
                        nc.vector.tensor_scalar_add(vt_tmp, ps[:, :CH], bvs)
                        for u in range(CH // P):
                            tp = sc.tile([P, 1024], f32, tag="sc")
                            nc.tensor.transpose(tp[:, :P], vt_tmp[:, u * P:(u + 1) * P], ident)
                            st = (CH // P) * tt + u
                            nc.vector.tensor_copy(vh[b][:, st, 0:64], tp[:, 0:64])
                            nc.vector.tensor_copy(vh[b][:, st, VSTRIDE:VSTRIDE + 64],
                                                  tp[:, 64:P])
                    yield

        def gen_vtrans(b):
            """Detached vhT -> ones-augmented vh transposes (b=1)."""
            for st in range(KT):
                tp = sc.tile([P, 1024], f32, tag="sc")
                nc.tensor.transpose(tp[:, :P], vhT1[:, st * P:(st + 1) * P], ident)
                nc.vector.tensor_copy(vh[b][:, st, 0:64], tp[:, 0:64])
                nc.vector.tensor_copy(vh[b][:, st, VSTRIDE:VSTRIDE + 64], tp[:, 64:P])
                yield

        def gen_T(b, j):
            """Transposed orientation + PV with ones-augmented V; produces
            ctxT columns for (b, j) and -ln(rowsums) for the natural pass."""
            h0 = 64 * j
            if ctxTs[b] is None:
                ctxTs[b] = ctxs_pool.tile([P, S], f32r, tag="ctxT", name=f"ctxT{b}")
            ctxT = ctxTs[b]
            nls = nlsp.tile([1, S], f32, tag="nls", name=f"nls{b}{j}")
            nlr = small.tile([P, QT], f32, tag="nlr", name=f"nlr{b}{j}")
            nlts[b][j] = nlr
            for st2 in range(2):
                cps = [ctxp.tile([65, 512], f32, tag="ctx", name=f"ctx{qc}")
                       for qc in range(2)]
                for kt in range(KT):
                    et = expp.tile([P, 1024], f32r, tag="expT")
                    pt = sc.tile([P, 1024], f32, tag="sc")
                    for qc2 in range(2):
                        qo = st2 * 1024 + qc2 * 512
                        nc.tensor.matmul(
                            pt[:, qc2 * 512:(qc2 + 1) * 512],
                            lhsT=khT[b][h0:h0 + 64, kt * P:(kt + 1) * P],
                            rhs=qhT[b][h0:h0 + 64, qo:qo + 512],
                            start=True, stop=True)
                    nc.scalar.activation(et, pt, Exp)
                    for qc2 in range(2):
                        nc.tensor.matmul(
                            cps[qc2],
                            lhsT=vh[b][:, kt, VSTRIDE * j: VSTRIDE * j + 65],
                            rhs=et[:, qc2 * 512:(qc2 + 1) * 512],
                            start=(kt == 0), stop=(kt == KT - 1))
                    yield
                # stripe flush: 1/rowsum feeds both the natural pass (via a
                # per-stripe DRAM-bounce reindex [1,1024] -> [128,8] on the
                # ACT ring) and the ctxT normalize
                so = st2 * 1024
                for qc2 in range(2):
                    qo = so + qc2 * 512
                    nc.vector.reciprocal(nls[0:1, qo:qo + 512], cps[qc2][64:65, :])
                scr = dscr.tile([S // 2], f32, tag="scr")
                nc.scalar.dma_start(scr.rearrange("(o n) -> o n", o=1),
                                    nls[0:1, so:so + 1024])
                nc.scalar.dma_start(nlr[:, st2 * 8:(st2 + 1) * 8],
                                    scr.rearrange("(t p) -> p t", p=P))
                for qc2 in range(2):
                    qo = so + qc2 * 512
                    rbc = small.tile([64, 512], f32, tag="rbc")
                    nc.gpsimd.partition_broadcast(rbc, nls[0:1, qo:qo + 512])
                    nc.vector.tensor_mul(ctxT[h0:h0 + 64, qo:qo + 512],
                                         cps[qc2][0:64, :], rbc)
                yield

        def gen_N(b, j, act_ring=False):
            """Natural orientation: exp on ACT, normalize by the T-side
            reciprocal rowsums on DVE. attn writes go on the ACT ring while
            input loads still own the SP ring, on the SP ring afterwards."""
            h0 = 64 * j
            nlr = nlts[b][j]
            dma = nc.scalar.dma_start if act_ring else nc.sync.dma_start
            for qt in range(QT):
                at = attnp.tile([P, S], f32, tag="attn")
                for kh in range(2):
                    pn = sc.tile([P, 1024], f32, tag="sc")
                    for kc2 in range(2):
                        ko = kh * 1024 + kc2 * 512
                        nc.tensor.matmul(
                            pn[:, kc2 * 512:(kc2 + 1) * 512],
                            lhsT=qhT[b][h0:h0 + 64, qt * P:(qt + 1) * P],
                            rhs=khT[b][h0:h0 + 64, ko:ko + 512],
                            start=True, stop=True)
                    nc.scalar.activation(at[:, kh * 1024:(kh + 1) * 1024], pn, Exp)
                nc.vector.tensor_scalar_mul(at, at, nlr[:, qt:qt + 1])
                dma(attn_d[b, j, qt * P:(qt + 1) * P, :], at)
                yield

        def gen_O(b):
            """Partial output projection (row-parallel), pout on the SP ring."""
            ctxT = ctxTs[b]
            for qt in range(QT):
                osb = attnp.tile([P, 1024], f32, tag="osb")
                ops = sc.tile([P, 1024], f32, tag="sc")
                for odc in range(2):
                    nc.tensor.matmul(ops[:, odc * 512:(odc + 1) * 512],
                                     lhsT=ctxT[:, qt * P:(qt + 1) * P],
                                     rhs=wo[:, odc * 512:(odc + 1) * 512],
                                     start=True, stop=True)
                nc.vector.tensor_copy(osb, ops)
                nc.sync.dma_start(pout_d[b * S + qt * P: b * S + (qt + 1) * P, :], osb)
                yield

        # ---- pipeline schedule ----
        # NOTE: emission order IS program order for Tile — every read must be
        # emitted after the write that produces its data. Interleave ratios
        # below are matched to per-unit production/consumption rates.
        p0 = gen_proj(0)
        for _ in range(12):         # chunks 0-3 of q,k,v before attention starts
            next(p0, None)
        # proj produces one 256-col chunk (2 key tiles) per 3 units; T consumes
        # 2 key tiles per 2 units -> 3:2 keeps writes ahead of reads
        t00 = gen_T(0, 0)
        p1 = gen_proj(1)
        _interleave((p0, 3), (t00, 2), stop_on_first=True)
        _interleave((t00, 2), (p1, 1), stop_on_first=True)
        _interleave((p1, 1), (gen_T(0, 1), 2), (gen_N(0, 0, act_ring=True), 1))
        vt1 = gen_vtrans(1)
        for _ in range(2):
            next(vt1, None)
        _interleave((vt1, 2), (gen_T(1, 0), 2), (gen_N(0, 1), 1))
        o0, o1 = gen_O(0), gen_O(1)
        _interleave((gen_T(1, 1), 2), (gen_N(1, 0), 1), (o0, 1))
        _interleave((gen_N(1, 1), 1), (o0, 1), (o1, 1))
        for _ in o1:
            pass

    nc.compile()
    return nc


def _prep_inputs(q, k, v, Wq, bq, Wk, bk, Wv, bv, Wo):
    """Host-side shard + layout prep. Returns per-core input maps."""
    scale = 1.0 / np.sqrt(np.float32(64.0))  # 0.125, exact
    qT = _round_f32r(np.ascontiguousarray(np.asarray(q, dtype=np.float32).reshape(NS, D).T))
    kT = _round_f32r(np.ascontiguousarray(np.asarray(k, dtype=np.float32).reshape(NS, D).T))
    vT = _round_f32r(np.ascontiguousarray(np.asarray(v, dtype=np.float32).reshape(NS, D).T))
    in_maps = []
    for c in range(NCORES):
        hs = slice(c * HD, (c + 1) * HD)
        in_maps.append({
            "qT": qT, "kT": kT, "vT": vT,
            "wqT": _round_f32r(np.ascontiguousarray((scale * Wq[hs, :]).T)),
            "wkT": _round_f32r(np.ascontiguousarray(Wk[hs, :].T)),
            "wvT": _round_f32r(np.ascontiguousarray(Wv[hs, :].T)),
            "woT": _round_f32r(np.ascontiguousarray(Wo[:, hs].T)),
            "bq": np.ascontiguousarray(scale * bq[hs], dtype=np.float32),
            "bk": np.ascontiguousarray(bk[hs], dtype=np.float32),
            "bv": np.ascontiguousarray(bv[hs], dtype=np.float32),
        })
    return in_maps


def kernel(q, k, v, Wq, bq, Wk, bk, Wv, bv, Wo, bo, *, _trace=False, _trace_kwargs=None):
    from concourse.bass_utils import run_bass_kernel_spmd

    if "nc" not in _cache:
        _cache["nc"] = build()
    nc = _cache["nc"]

    in_maps = _prep_inputs(q, k, v, Wq, bq, Wk, bk, Wv, bv, Wo)
    res = run_bass_kernel_spmd(nc, in_maps, core_ids=list(range(NCORES)),
                               trace=_trace, **(_trace_kwargs or {}))
    _cache["last_result"] = res

    attn = np.empty((B, 16, S, S), dtype=np.float32)
    acc = np.zeros((NS, D), dtype=np.float64)
    for c in range(NCORES):
        r = res.results[c]
        attn[:, 2 * c: 2 * c + 2] = r["attn"]
        acc += r["pout"].astype(np.float64)
    out = (acc + np.asarray(bo, dtype=np.float64)).astype(np.float32).reshape(B, S, D)
    return (out, attn)
